# revision 35
# baseline (speedup 1.0000x reference)
"""Trainium2 Bass kernel for nn_MultiHeadAttention_76295799046818.

Multi-head attention: B=2, S=2048, D=1024, H=16 heads (d_k=64), causal mask,
fp32 reference.  Sharded over 8 NeuronCores as data-parallel over batch (2) x
tensor-parallel over heads (4 heads per core).  Wq/Wk/Wv are column-parallel;
Wo is row-parallel and each core emits its partial output (its 4 heads'
contribution to the full [S, D] output); the host sums the 4 partials per
batch (the unshard step), avoiding the on-device all-reduce.

v2: software-pipelined single pass.  The baseline ran proj -> attention ->
tail as three serial phases; the ACT engine (softmax exp, ~92us) sat idle for
the first 70us and the PE idled in the tail.  v2 interleaves per q-chunk:

  proj(0) | attn(0)+weave[proj(1)] | attn(1)+weave[proj(2),tail(0)] | ...

so PE, ACT, DVE and GPSIMD run concurrently throughout.  Other changes vs
the baseline:
  - causal mask applied as a post-exp 0/1 multiply on GPSIMD (was: identity
    matmul NEG-add on the PE) - saves ~8.5us of PE and removes a PSUM pass.
  - even AND odd heads' AV accumulate simultaneously into two PSUM banks
    (was: odd head re-read saved exp tiles in a second burst).
  - softmax denominators leave PSUM by direct DMA (pav row -> SBUF rden_raw),
    replacing one-hot gather matmuls + DVE row copies; reciprocal is the ~5x
    faster reciprocal_approx_fast (51-ULP, plenty for denominators), done
    per q-chunk so the tail pipelines instead of waiting a global sync.
  - exp is trimmed to the group's causal lower bound (skips fully-masked
    columns), cutting ACT work ~10%.

Device pipeline per core (bf16 matmuls, fp32 PSUM accumulation):
  1. Q^T, K^T projections in [d_k-pair, S] layout (per-partition bias add),
     V in [S, d_k] layout: vaug_e = [V_even | ones] (65 cols), vaug_o =
     [ones | 0 | V_odd] (128 cols) so the ones column doubles as the softmax
     denominator row.
  2. Per (head-pair, q-chunk of 512): S^T = K^T.T Q^T block-matmuls; the two
     heads of a pair sit at SBUF partitions 0-63 / 64-127, so their K=64
     matmuls occupy disjoint PE row-groups and run concurrently.  One ACT
     exp (scale=1/sqrt(d_k) folded in) per 2 k-tiles -> bf16; GPSIMD zeroes
     the causal triangle of diagonal tiles post-exp.
  3. AV: pav_e[65,512] (C^T_e rows 0-63, den_e row 64) and pav_o[128,512]
     (den_o row 0, C^T_o rows 64-127) accumulate in parallel banks.
  4. Tail per q-chunk: dens DMA'd to rden_raw[4,512], approx-reciprocal,
     PE broadcast (bsel one-hot), DVE normalize, partial Wo matmul, bias,
     DMA out.  Tail(qc) is woven into attn(qc+1)'s PE stream.

NOTE: assumes every q column of the first kept k-tile of each chunk is
valid (true for causal); a fully-masked q-chunk is unsupported.
"""

import numpy as np
import ml_dtypes

import concourse.bass as bass
import concourse.mybir as mybir
import concourse.tile as tile
from concourse import bacc
from concourse.bass_utils import run_bass_kernel_spmd

BF16 = ml_dtypes.bfloat16

B, S, D, H, DK = 2, 2048, 1024, 16, 64
N_CORES = 8
TP = 4  # head-parallel degree (per batch)
HPC = H // TP  # heads per core = 4
O = HPC * DK  # output channels per core = 256
KT_BLK = 128
QT_BLK = 512
N_KT = S // KT_BLK  # 16
N_QC = S // QT_BLK  # 4
KC = D // 128  # 8 contraction chunks for projections

_CACHE = {}


def _mask_structure(mask):
    """Classify [KT_BLK x QT_BLK] blocks of the S^T mask.

    Returns (keep[ki][qc] in {'full','skip',int}, keep01 [n,128,512] f32
    {0,1} multiplicative patterns, ranges[ki][qc] = (qk_lo, m_hi)): an int
    indexes keep01, qk_lo is the first q column with any kept element (QK/AV
    skip columns below it), m_hi is one past the last q column with any
    dropped element (the 0/1 multiply covers [qk_lo, m_hi)).
    """
    dropped = np.asarray(mask) == 0
    keep, ranges = [], []
    tiles = []
    tile_index = {}
    for ki in range(N_KT):
        row, rrow = [], []
        for qc in range(N_QC):
            sub = dropped[qc * QT_BLK:(qc + 1) * QT_BLK,
                          ki * KT_BLK:(ki + 1) * KT_BLK].T  # [128, 512]
            if not sub.any():
                row.append("full")
                rrow.append((0, 0))
            elif sub.all():
                row.append("skip")
                rrow.append((0, 0))
            else:
                key = sub.tobytes()
                if key not in tile_index:
                    tile_index[key] = len(tiles)
                    tiles.append(np.where(sub, 0.0, 1.0).astype(np.float32))
                row.append(tile_index[key])
                col_kept = ~sub.all(axis=0)
                col_drop = sub.any(axis=0)
                qk_lo = int(np.argmax(col_kept))
                m_hi = int(QT_BLK - np.argmax(col_drop[::-1]))
                rrow.append((qk_lo, m_hi))
        keep.append(row)
        ranges.append(rrow)
    if not tiles:
        tiles.append(np.ones((KT_BLK, QT_BLK), np.float32))
    return keep, np.stack(tiles), ranges


def _build(keep, n_mixed, ranges):
    nc = bacc.Bacc("TRN2", target_bir_lowering=False, debug=False,
                   num_devices=N_CORES)
    dt = mybir.dt
    f32, bf16, f32r = dt.float32, dt.bfloat16, dt.float32r

    def din(name, shape, dtype=bf16):
        return nc.dram_tensor(name, shape, dtype, kind="ExternalInput").ap()

    # all DRAM inputs are pre-packed partition-major on the host so every
    # dma_start is a plain dense copy
    xqt_d = din("xqt", [N_QC, 128, KC, QT_BLK])
    xkt_d = din("xkt", [N_QC, 128, KC, QT_BLK])
    xvt_d = din("xvt", [N_QC, 128, KC, QT_BLK])
    wqt_d = din("wqt", [128, KC, O])
    wkt_d = din("wkt", [128, KC, O])
    wvt_d = din("wvt", [128, KC, O])
    wot_d = din("wot", [128, 2, D])
    bq_d = din("bqc", [128, 2], f32)
    bk_d = din("bkc", [128, 2], f32)
    bvb_d = din("bvb", [128, O], f32)
    bo_d = din("boc", [128, 8], f32)
    um_d = din("um01", [128, n_mixed, QT_BLK])
    # [qc, j-pair, partition, j-half, col]: 2KB contiguous per partition
    # per DMA (osb pairs), the best DMA line size the tail can produce
    out_d = nc.dram_tensor("out", [N_QC, 4, 128, 2, QT_BLK], bf16,
                           kind="ExternalOutput").ap()

    EXPF = mybir.ActivationFunctionType.Exp

    with tile.TileContext(nc) as tc:
        with (
            tc.tile_pool(name="const", bufs=1) as cpool,
            tc.tile_pool(name="xin", bufs=3) as xpool,
            tc.tile_pool(name="expp", bufs=6) as epool,
            tc.tile_pool(name="ctp", bufs=2) as ctpool,
            tc.tile_pool(name="rdp", bufs=4) as rpool,
            tc.tile_pool(name="outp", bufs=4) as opool,
            tc.tile_pool(name="ps", bufs=2, space="PSUM") as ps,
        ):
            # warm-up source: GPSIMD memset needs no DMA, so the PE can run
            # dummy matmuls through the initial DMA dead zone, keeping the
            # HAM clock gate at 8/8 when the real work arrives
            ones_sb = cpool.tile([128, 512], bf16, name="ones_sb")
            nc.gpsimd.memset(ones_sb[:], 1.0)
            # hot-path inputs first, spread across engine queues so the
            # descriptor issue isn't serialized on the Sync queue
            wk_sb = cpool.tile([128, KC, O], bf16, name="wk_sb")
            nc.scalar.dma_start(wk_sb[:, 0:1, :], wkt_d[:, 0:1, :])
            nc.gpsimd.dma_start(wk_sb[:, 1:4, :], wkt_d[:, 1:4, :])
            nc.sync.dma_start(wk_sb[:, 4:8, :], wkt_d[:, 4:8, :])
            xk0 = xpool.tile([128, KC, QT_BLK], bf16, name="xk", tag="xk")
            nc.scalar.dma_start(xk0[:, 0:1, :], xkt_d[0][:, 0:1, :])
            nc.gpsimd.dma_start(xk0[:, 1:4, :], xkt_d[0][:, 1:4, :])
            nc.sync.dma_start(xk0[:, 4:8, :], xkt_d[0][:, 4:8, :])
            wq_sb = cpool.tile([128, KC, O], bf16, name="wq_sb")
            nc.gpsimd.dma_start(wq_sb[:, 0:4, :], wqt_d[:, 0:4, :])
            nc.scalar.dma_start(wq_sb[:, 4:8, :], wqt_d[:, 4:8, :])
            xq0 = xpool.tile([128, KC, QT_BLK], bf16, name="xq", tag="xq")
            nc.scalar.dma_start(xq0[:, 0:4, :], xqt_d[0][:, 0:4, :])
            nc.gpsimd.dma_start(xq0[:, 4:8, :], xqt_d[0][:, 4:8, :])
            wv_sb = cpool.tile([128, KC, O], bf16, name="wv_sb")
            nc.sync.dma_start(wv_sb[:], wvt_d[:])
            xv0 = xpool.tile([128, KC, QT_BLK], bf16, name="xv", tag="xv")
            nc.sync.dma_start(xv0[:], xvt_d[0])
            def warm(n):
                """n dummy matmuls: fill DMA-wait bubbles in the early
                phase so the HAM clock gate never re-throttles."""
                for _ in range(n):
                    wt = ps.tile([128, QT_BLK], f32, name="warm", tag="pp",
                                 bufs=2)
                    nc.tensor.matmul(wt[0:64, :], ones_sb[:, 0:64],
                                     ones_sb[:], start=True, stop=True,
                                     skip_group_check=True)

            # PE warm-up: ~24 x 512-free dummy matmuls bridge the DMA wait
            warm(24)
            bq_sb = cpool.tile([128, 2], f32, name="bq_sb")
            nc.sync.dma_start(bq_sb[:], bq_d[:])
            bk_sb = cpool.tile([128, 2], f32, name="bk_sb")
            nc.sync.dma_start(bk_sb[:], bk_d[:])
            bvb_sb = cpool.tile([128, O], f32, name="bvb_sb")
            nc.sync.dma_start(bvb_sb[:], bvb_d[:])
            um_sb = cpool.tile([128, n_mixed, QT_BLK], bf16, name="um_sb")
            nc.sync.dma_start(um_sb[:], um_d[:])
            # tail-phase constants (loaded after chunk-1 prefetch below;
            # first use is tail(0), woven into attn(1))
            wo_sb = cpool.tile([128, 2, D], bf16, name="wo_sb")
            bo_sb = cpool.tile([128, 8], f32, name="bo_sb")


            qt_sb = cpool.tile([128, 2, S], bf16, name="qt_sb")
            kt_sb = cpool.tile([128, 2, S], bf16, name="kt_sb")
            vaug_e = cpool.tile([128, N_KT, 2, 66], bf16, name="vaug_e")
            nc.gpsimd.memset(vaug_e[:], 1.0)
            vaug_o = cpool.tile([128, N_KT, 2, 128], bf16, name="vaug_o")
            nc.gpsimd.memset(vaug_o[:], 0.0)
            nc.gpsimd.memset(vaug_o[:, :, :, 0:1], 1.0)

            # ---- emission helpers -------------------------------------
            IDF = mybir.ActivationFunctionType.Identity

            def proj_items(sc, xq, xk, xv):
                """PE work items (closures) for projecting chunk sc.
                Returns (kq_items, v_items).  For chunk 0 the K/Q bias adds
                go to the Scalar engine (idle until the first exp)."""
                ssl = bass.ds(sc * QT_BLK, QT_BLK)

                def bias_add(dst, src, bias):
                    if sc == 0:
                        nc.scalar.activation(dst, src, IDF, bias=bias)
                    else:
                        nc.vector.tensor_scalar(dst, src, bias, None,
                                                mybir.AluOpType.add)

                def mk_k(ot):
                    def it():
                        osl = bass.ds(ot * 128, 128)
                        pk = ps.tile([128, QT_BLK], f32, name="pk", tag="pp",
                                     bufs=2)
                        for kc in range(KC):
                            nc.tensor.matmul(pk[:], wk_sb[:, kc, osl],
                                             xk[:, kc, :], start=(kc == 0),
                                             stop=(kc == KC - 1))
                        bias_add(kt_sb[:, ot, ssl], pk[:],
                                 bk_sb[:, ot:ot + 1])
                    return it

                def mk_v(mt):
                    def it():
                        pv = ps.tile([128, QT_BLK], f32, name="pv", tag="pp",
                                     bufs=2)
                        for kc in range(KC):
                            nc.tensor.matmul(
                                pv[:, 0:O], xv[:, kc, bass.ds(mt * 128, 128)],
                                wv_sb[:, kc, :], start=(kc == 0),
                                stop=(kc == KC - 1))
                        pvr = pv[:, 0:O].rearrange(
                            "p (hp two d) -> p hp two d", hp=2, two=2)
                        bvr = bvb_sb[:].rearrange(
                            "p (hp two d) -> p hp two d", hp=2, two=2)
                        nc.vector.tensor_tensor(
                            vaug_e[:, sc * 4 + mt, :, 0:64],
                            pvr[:, :, 0, :], bvr[:, :, 0, :],
                            mybir.AluOpType.add)
                        nc.vector.tensor_tensor(
                            vaug_o[:, sc * 4 + mt, :, 64:128],
                            pvr[:, :, 1, :], bvr[:, :, 1, :],
                            mybir.AluOpType.add)
                    return it

                def mk_q(ot):
                    def it():
                        osl = bass.ds(ot * 128, 128)
                        pq = ps.tile([128, QT_BLK], f32, name="pq", tag="pp",
                                     bufs=2)
                        for kc in range(KC):
                            nc.tensor.matmul(pq[:], wq_sb[:, kc, osl],
                                             xq[:, kc, :], start=(kc == 0),
                                             stop=(kc == KC - 1))
                        bias_add(qt_sb[:, ot, ssl], pq[:],
                                 bq_sb[:, ot:ot + 1])
                    return it

                kq = [mk_k(0), mk_k(1), mk_q(0), mk_q(1)]
                vs = [mk_v(mt) for mt in range(4)]
                return kq, vs

            def tail_items(qc, ctraw, rdens):
                """PE/DVE work items for the tail of chunk qc."""
                qsl = bass.ds(qc * QT_BLK, QT_BLK)
                ct_sb = ctpool.tile([128, 2, QT_BLK], bf16, name="ct",
                                    tag="ct")
                items = []

                def mk_norm(hp):
                    def it():
                        # broadcast raw dens: partitions 0-63 <- den_even
                        # (stg row 64), 64-127 <- den_odd (stg row 0);
                        # then one full-lane approx reciprocal
                        stg = rdens[hp]
                        pbc = ps.tile([128, QT_BLK], f32, name="pbc",
                                      tag="pp", bufs=2)
                        nc.tensor.matmul(pbc[0:64, :], ones_sb[64:65, 0:64],
                                         stg[64:65, :], start=True,
                                         stop=True, skip_group_check=True)
                        nc.tensor.matmul(pbc[64:128, :], ones_sb[0:1, 0:64],
                                         stg[0:1, :], start=True,
                                         stop=True, skip_group_check=True)
                        pbci = rpool.tile([128, QT_BLK], f32, name="pbci",
                                          tag="pbci", bufs=2)
                        nc.vector.reciprocal_approx_fast(pbci[:], pbc[:])
                        nc.vector.tensor_tensor(ct_sb[:, hp, :],
                                                ctraw[:, hp, :], pbci[:],
                                                mybir.AluOpType.mult)
                    return it

                osb_pair = [None]

                def mk_wo(jt):
                    def it():
                        pwo = ps.tile([128, QT_BLK], f32, name="pwo",
                                      tag="pp", bufs=2)
                        for kc in range(2):
                            nc.tensor.matmul(
                                pwo[:], wo_sb[:, kc, bass.ds(jt * 128, 128)],
                                ct_sb[:, kc, :], start=(kc == 0),
                                stop=(kc == 1))
                        if jt % 2 == 0:
                            osb_pair[0] = opool.tile([128, 2, QT_BLK], bf16,
                                                     name="osb", tag="osb")
                        osb = osb_pair[0]
                        half = jt % 2
                        # last chunk: split bias adds with the (by then
                        # idle) Scalar engine so the terminal tail drains
                        # faster
                        if qc == N_QC - 1 and jt % 2 == 0:
                            nc.scalar.activation(osb[:, half, :], pwo[:],
                                                 IDF,
                                                 bias=bo_sb[:, jt:jt + 1])
                        else:
                            nc.vector.tensor_scalar(osb[:, half, :], pwo[:],
                                                    bo_sb[:, jt:jt + 1],
                                                    None,
                                                    mybir.AluOpType.add)
                        if half == 1:
                            nc.sync.dma_start(out_d[qc][jt // 2], osb[:])
                    return it

                for hp in range(2):
                    items.append(mk_norm(hp))
                for jt in range(8):
                    items.append(mk_wo(jt))
                return items

            def attn_emit(qc, backlog, pre_av=()):
                """Emit attention for chunk qc, weaving backlog items (PE
                work for the next chunk's projections and the previous
                chunk's tail) between QK/AV groups to keep the PE dense
                while ACT chews on exp.  pre_av items are emitted after the
                first group's exp but before any AV (used for chunk 0's V
                projections, which the AVs consume)."""
                kis = [ki for ki in range(N_KT) if keep[ki][qc] != "skip"]
                assert kis, "fully-masked q-chunk unsupported"
                groups = [kis[i:i + 2] for i in range(0, len(kis), 2)]
                n_steps = 2 * len(groups)
                ctraw = ctpool.tile([128, 2, QT_BLK], bf16, name="ctraw",
                                    tag="ctraw")
                rdens = []
                step = 0
                emitted = 0
                pre_av = list(pre_av)
                for hp in range(2):
                    pav_e = ps.tile([65, QT_BLK], f32, name="pav_e",
                                    tag="pav", bufs=2)
                    pav_o = ps.tile([128, QT_BLK], f32, name="pav_o",
                                    tag="pav", bufs=2)
                    n_av = 0
                    for g in groups:
                        st2 = [ps.tile([128, 2, QT_BLK], f32, name="st",
                                       tag="st", bufs=2)
                               for _ in range(2)]
                        et2 = [epool.tile([128, 2, QT_BLK], bf16,
                                          name="et", tag="et")
                               for _ in range(2)]
                        glo = []
                        for gi, ki in enumerate(g):
                            qk_lo = 0
                            if keep[ki][qc] != "full":
                                qk_lo = ranges[ki][qc][0]
                            glo.append(qk_lo)
                            for side in range(2):
                                po = bass.ds(side * 64, 64)
                                nc.tensor.matmul(
                                    st2[side][:, gi, qk_lo:QT_BLK],
                                    kt_sb[po, hp,
                                          bass.ds(ki * KT_BLK, KT_BLK)],
                                    qt_sb[po, hp,
                                          bass.ds(qc * QT_BLK + qk_lo,
                                                  QT_BLK - qk_lo)],
                                    start=True, stop=True)
                        gmin = min(glo)
                        for side in range(2):
                            if len(g) == 2:
                                nc.scalar.activation(
                                    et2[side][:, :, gmin:QT_BLK],
                                    st2[side][:, :, gmin:QT_BLK],
                                    EXPF, scale=0.125)
                            else:
                                nc.scalar.activation(
                                    et2[side][:, 0, gmin:QT_BLK],
                                    st2[side][:, 0, gmin:QT_BLK],
                                    EXPF, scale=0.125)
                        while pre_av:
                            pre_av.pop(0)()
                        # zero the dropped region of mixed tiles (post-exp)
                        for gi, ki in enumerate(g):
                            if keep[ki][qc] in ("full", "skip"):
                                continue
                            lo, m_hi = ranges[ki][qc]
                            idx = keep[ki][qc]
                            for side in range(2):
                                nc.gpsimd.tensor_tensor(
                                    et2[side][:, gi, lo:m_hi],
                                    et2[side][:, gi, lo:m_hi],
                                    um_sb[:, idx, lo:m_hi],
                                    mybir.AluOpType.mult)
                        # AV for both heads streams with the groups
                        for gi, ki in enumerate(g):
                            av_lo = 0
                            if n_av > 0 and keep[ki][qc] != "full":
                                av_lo = ranges[ki][qc][0]
                            nc.tensor.matmul(
                                pav_e[:, av_lo:QT_BLK],
                                vaug_e[:, ki, hp, 0:65],
                                et2[0][:, gi, av_lo:QT_BLK],
                                start=(n_av == 0),
                                stop=(n_av == len(kis) - 1),
                                skip_group_check=True)
                            nc.tensor.matmul(
                                pav_o[:, av_lo:QT_BLK],
                                vaug_o[:, ki, hp, :],
                                et2[1][:, gi, av_lo:QT_BLK],
                                start=(n_av == 0),
                                stop=(n_av == len(kis) - 1),
                                skip_group_check=True)
                            n_av += 1
                        # weave in backlog (linear pacing)
                        step += 1
                        want = step * len(backlog) // n_steps
                        while emitted < want:
                            backlog[emitted]()
                            emitted += 1
                    # raw dens out of PSUM: stg row 64 = den_even (lane-
                    # aligned with pav_e row 64), row 0 = den_odd
                    stg = rpool.tile([65, QT_BLK], bf16, name="stg",
                                     tag="stg", bufs=4)
                    nc.vector.tensor_copy(stg[64:65, :], pav_e[64:65, :])
                    nc.vector.tensor_copy(stg[0:1, :], pav_o[0:1, :])
                    rdens.append(stg)
                    nc.vector.tensor_copy(ctraw[0:64, hp, :], pav_e[0:64, :])
                    nc.vector.tensor_copy(ctraw[64:128, hp, :],
                                          pav_o[64:128, :])
                while emitted < len(backlog):
                    backlog[emitted]()
                    emitted += 1
                return ctraw, rdens

            # ---- pipelined schedule -----------------------------------
            xs = {0: (xq0, xk0, xv0)}

            def fetch_chunk(sc):
                xq = xpool.tile([128, KC, QT_BLK], bf16, name="xq", tag="xq")
                nc.sync.dma_start(xq[:], xqt_d[sc])
                xk = xpool.tile([128, KC, QT_BLK], bf16, name="xk", tag="xk")
                nc.sync.dma_start(xk[:, 0:3, :], xkt_d[sc][:, 0:3, :])
                nc.sync.dma_start(xk[:, 3:8, :], xkt_d[sc][:, 3:8, :])
                xv = xpool.tile([128, KC, QT_BLK], bf16, name="xv", tag="xv")
                nc.sync.dma_start(xv[:], xvt_d[sc])
                xs[sc] = (xq, xk, xv)

            fetch_chunk(1)  # prefetch distance 2 via xpool bufs=3
            nc.sync.dma_start(wo_sb[:], wot_d[:])
            nc.sync.dma_start(bo_sb[:], bo_d[:])
            kq0, v0 = proj_items(0, *xs[0])
            for it in kq0:
                it()
                warm(2)  # DMA-starved era: keep the PE warm between bursts
            prev_tail = None
            for qc in range(N_QC):
                if qc + 2 < N_QC:
                    fetch_chunk(qc + 2)
                backlog = []
                if qc + 1 < N_QC:
                    kq, vs = proj_items(qc + 1, *xs[qc + 1])
                    backlog += kq + vs
                if qc == 0:
                    # chunk 0's attention is DMA-starved: pad the weave
                    # with warm-up fillers so the PE never idles >3.4us
                    wv_items = [lambda: warm(2) for _ in range(6)]
                    backlog = [x for pair in
                               zip(wv_items, backlog[:6]) for x in pair] \
                        + backlog[6:]
                if prev_tail is not None:
                    # alternate proj/tail items so neither starves
                    a, b = backlog, prev_tail
                    backlog = []
                    for i in range(max(len(a), len(b))):
                        if i < len(a):
                            backlog.append(a[i])
                        if i < len(b):
                            backlog.append(b[i])
                ctraw, rdens = attn_emit(qc, backlog,
                                         pre_av=v0 if qc == 0 else ())
                prev_tail = tail_items(qc, ctraw, rdens)
            for it in prev_tail:
                it()

    nc.compile()
    return nc


def kernel(query, key, value, mask, Wq, bq, Wk, bk, Wv, bv, Wo, bo):
    query = np.asarray(query, np.float32)
    key_ = np.asarray(key, np.float32)
    value = np.asarray(value, np.float32)
    Wq, Wk, Wv, Wo = (np.asarray(w, np.float32) for w in (Wq, Wk, Wv, Wo))
    bq, bk, bv, bo = (np.asarray(b_, np.float32) for b_ in (bq, bk, bv, bo))

    keep, mtiles, ranges = _mask_structure(mask)
    ckey = np.asarray(mask).tobytes()
    if ckey not in _CACHE:
        _CACHE.clear()
        _CACHE[ckey] = _build(keep, len(mtiles), ranges)
    nc = _CACHE[ckey]

    def xt(x):  # [S, D] -> [N_QC, 128, KC, QT_BLK] bf16, partition-major
        a = x.T.reshape(KC, 128, S).transpose(1, 0, 2)  # [128, KC, S]
        a = a.reshape(128, KC, N_QC, QT_BLK).transpose(2, 0, 1, 3)
        return np.ascontiguousarray(a).astype(BF16)

    def wslice(W, c):  # [D, D] -> [128, KC, O] bf16 of W[o_slice].T
        hg = c % TP
        a = W[hg * O:(hg + 1) * O].T.reshape(KC, 128, O).transpose(1, 0, 2)
        return np.ascontiguousarray(a).astype(BF16)

    um_pm = np.ascontiguousarray(mtiles.transpose(1, 0, 2))  # [128, n, 512]

    in_maps = []
    for c in range(N_CORES):
        b_, hg = c // TP, c % TP
        osl = slice(hg * O, (hg + 1) * O)
        bo_part = bo if hg == 0 else np.zeros_like(bo)
        wot = Wo[:, osl].T.reshape(2, 128, D).transpose(1, 0, 2)
        in_maps.append({
            "xqt": xt(query[b_]),
            "xkt": xt(key_[b_]),
            "xvt": xt(value[b_]),
            "wqt": wslice(Wq, c),
            "wkt": wslice(Wk, c),
            "wvt": wslice(Wv, c),
            "wot": np.ascontiguousarray(wot).astype(BF16),
            "bqc": np.ascontiguousarray(bq[osl].reshape(2, 128).T),
            "bkc": np.ascontiguousarray(bk[osl].reshape(2, 128).T),
            "bvb": np.ascontiguousarray(np.broadcast_to(bv[osl], (128, O))),
            "boc": np.ascontiguousarray(bo_part.reshape(8, 128).T),
            "um01": um_pm.astype(BF16),
        })

    res = run_bass_kernel_spmd(nc, in_maps, core_ids=list(range(N_CORES)))

    out = np.zeros((B, S, D), np.float32)
    for c in range(N_CORES):
        # [qc, jp, p, h, col] -> out^T[j, s], j=(2*jp+h)*128+p, s=qc*512+col
        a = res.results[c]["out"].reshape(N_QC, 4, 128, 2, QT_BLK)
        part = a.transpose(1, 3, 2, 0, 4).reshape(D, S)
        out[c // TP] += part.T.astype(np.float32)
    return out
